# revision 34
# baseline (speedup 1.0000x reference)
"""Trainium2 Bass kernel for ExpBertSelfAttention (B=2, S=2048, D=1024, H=16).

Sharding: 8 cores; core c handles batch b=c//4 and 4 consecutive heads
4*(c%4)..4*(c%4)+3 (data-parallel on B, tensor-parallel on heads).  The dense
output projection is row-parallel: each core returns a partial [S, D] sum
(bf16); the host adds the 4 partials per batch + b_dense.

Per-core data path (all matmuls bf16 with f32 PSUM; 1/sqrt(hd) folded into
Wq on host; softmax without max-subtraction — scores are O(1) by
construction):

  - QKV: qkvT[768, S] = wqkv^T hsT.  Pair-0 m-tiles run in the lead-in;
    pair-1 runs as 8-matmul accumulation chains interleaved into the
    attention kt loops (PE fill work), both through a shared 3-slot
    [128, 1024] PSUM ring.  Drains: ACT (lead-in) / DVE (in-flight).
  - V is transposed to [key, hd] via XBAR DMA transpose ([64,128]->[128,64]
    into 80-element slots: dst offsets must stay 32B-aligned), with a
    constant ones column at index 64 so PV also emits the softmax row-sum.
  - Attention is processed one head at a time (8 sections = 4 heads x 2
    q-chunks); the ctx accumulator [65, 1024] then needs only 2 PSUM banks.
    Per kt: QK (2 x N=512 matmuls into a ring slot), one [128,1024] exp on
    ACT (psum f32 -> sbuf bf16), multiplicative {0,1} bf16 mask on DVE (2x
    16-bit rate) or GPSIMD (1 in 3, SBUF-only), PV (2 x N=512 into ctx).
  - normalize: row-sum row 64 -> ACT copy to SBUF -> k=1 matmul broadcast
    across partitions (into a ring slot) -> reciprocal_approx_fast (DVE) ->
    multiply into the pair-stacked bf16 ctx_pair; odd heads reach
    partitions 64-127 via a small partition-shifting SBUF->SBUF DMA.
  - dense: y[qtile, D] accumulated over the 2 pairs through ring slots,
    DVE-drained to bf16, streamed out per qtile; the qc-0 half overlaps the
    qc-1 attention.

Engine budget per core (TimelineSim model): PE.ENGINE ~167us (401k rows
@2.4GHz) is the roof; PE.SEQ ~144us (880 Ldweights+Matmult pairs), ACT
~141us (128 exps + row copies), DVE ~107us, Pool ~87us, DMA ~62us fit under.
"""

import os
import sys

for _p in ("/opt/trn_rl_repo", "/root/.axon_site/_ro/trn_rl_repo"):
    if os.path.isdir(_p) and _p not in sys.path:
        sys.path.insert(0, _p)

import numpy as np
import ml_dtypes

import concourse.bass as bass
import concourse.tile as tile
from concourse import bacc, mybir
from concourse import bass_utils

B, S, D, H = 2, 2048, 1024, 16
HD = D // H  # 64
SCALE = float(np.sqrt(HD).astype(np.float32))
NCORES = 8
HPC = H // (NCORES // B)  # heads per core = 4
P = 128
F32 = mybir.dt.float32
BF16 = mybir.dt.bfloat16
AF = mybir.ActivationFunctionType

KT_HS = D // P            # 8 contraction tiles for QKV
KT_S = S // P             # 16 key tiles for attention
QC = 1024                 # q chunk
NQC = S // QC             # 2
NQT = QC // P             # 8 query tiles per chunk
VW = 80                   # v_sb slot stride (65 used: 64 v + ones column);
                          # XBAR-transpose dst offsets must be 32B-aligned


def build_program():
    nc = bacc.Bacc("TRN2", target_bir_lowering=False, debug=False,
                   num_devices=NCORES)

    hsT = nc.dram_tensor("hsT", [D, S], BF16, kind="ExternalInput").ap()
    wqkv = nc.dram_tensor("wqkv", [D, 3 * HPC * HD], BF16,
                          kind="ExternalInput").ap()
    bqkv = nc.dram_tensor("bqkv", [3 * HPC * HD], F32,
                          kind="ExternalInput").ap()
    maskT = nc.dram_tensor("maskT", [S, S], BF16, kind="ExternalInput").ap()
    wd = nc.dram_tensor("wd", [2 * P, D], BF16, kind="ExternalInput").ap()
    y = nc.dram_tensor("y", [S, D], BF16, kind="ExternalOutput").ap()
    dbg = os.environ.get("BK_DEBUG", "") == "1"
    if dbg:
        d_qkvT = nc.dram_tensor("d_qkvT", [P, 6, S], BF16,
                                kind="ExternalOutput").ap()
        d_v = nc.dram_tensor("d_v", [P, HPC, KT_S, VW], BF16,
                             kind="ExternalOutput").ap()
        d_cp = nc.dram_tensor("d_cp", [P, 2, S], BF16,
                              kind="ExternalOutput").ap()

    hsT_r = hsT.rearrange("(t p) n -> p t n", p=P)
    w_r = wqkv.rearrange("(t p) n -> p t n", p=P)

    with tile.TileContext(nc) as tc:
        with tc.tile_pool(name="persist", bufs=1) as persist:
            hsT_sb = persist.tile([P, KT_HS, S], BF16)          # 32 KB/part
            w_sb = persist.tile([P, KT_HS, 3 * HPC * HD], BF16)  # 12 KB/part
            bq_sb = persist.tile([P, 6], F32)
            qkvT = persist.tile([P, 6, S], BF16)                # 24 KB/part
            # v slots [h, kt, VW]: cols 0-63 = V^T, col 64 = ones
            v_sb = persist.tile([P, HPC, KT_S, VW], BF16)       # 10 KB/part
            wd_sb = persist.tile([P, 2, D], BF16)               # 4 KB/part
            ctx_pair = persist.tile([P, 2, S], BF16)            # 8 KB/part
            ones_f = persist.tile([P, HD], F32)
            ones_bf = persist.tile([P, HD], BF16)

            nc.sync.dma_start(bq_sb[:], bqkv.rearrange("(t p) -> p t", p=P))
            nc.vector.memset(ones_f[:], 1.0)
            nc.vector.tensor_copy(ones_bf[:], ones_f[:])
            nc.vector.tensor_copy(
                v_sb[:, :, :, HD:HD + 1].rearrange("p a b c -> p (a b c)"),
                ones_f[:, 0:HPC * KT_S])

            with (
                tc.tile_pool(name="mp", bufs=2) as mp,
                tc.tile_pool(name="ptp", bufs=6) as ptp,
                tc.tile_pool(name="rp", bufs=2) as rp,
                tc.tile_pool(name="yp", bufs=2) as yp,
                tc.tile_pool(name="sps", bufs=3, space="PSUM") as sps,
                tc.tile_pool(name="cps", bufs=1, space="PSUM") as cps,
            ):
                # ---------------- Phase 1: QKV pair 0 ----------------
                # (w_kt, hsT_kt cols 0:512) DMA pairs stream first so the
                # first accumulation chains are DMA-complete early.
                for kt in range(KT_HS):
                    nc.sync.dma_start(w_sb[:, kt, :], w_r[:, kt, :])
                    nc.sync.dma_start(hsT_sb[:, kt, 0:512],
                                      hsT_r[:, kt, 0:512])
                # first key tiles of the qc-0 mask sneak in ahead of the
                # hsT second halves so attention isn't mask-gated at start
                mask0 = mp.tile([P, KT_S, QC], BF16, tag="mask",
                                name="mask_qc0")
                for g in range(2):
                    nc.sync.dma_start(
                        mask0[:, 2 * g:2 * g + 2, :],
                        maskT[g * 256:(g + 1) * 256, 0:QC].rearrange(
                            "(t p) q -> p t q", p=P))
                for kt in range(KT_HS):
                    nc.sync.dma_start(hsT_sb[:, kt, 512:S],
                                      hsT_r[:, kt, 512:S])
                for g in range(2, 8):
                    nc.sync.dma_start(
                        mask0[:, 2 * g:2 * g + 2, :],
                        maskT[g * 256:(g + 1) * 256, 0:QC].rearrange(
                            "(t p) q -> p t q", p=P))

                def qkv_chain(mt, nch, drain, half=None):
                    """one [128,512] column chunk of qkvT m-tile mt through
                    a ring slot; drain = 'act' (lead-in) or 'dve'.
                    half: None = whole 8-kt chain; (state, 0)/(state, 1) =
                    4-kt halves (keeps each PE burst under the exp period
                    when interleaved into attention)."""
                    cs = slice(nch * 512, (nch + 1) * 512)
                    if half is None or half[1] == 0:
                        ps = sps.tile([P, 512], F32, tag="s",
                                      name=f"qkv_ps{mt}_{nch}")
                        if half is not None:
                            half[0]["ps"] = ps
                        kts = range(KT_HS) if half is None else range(4)
                    else:
                        ps = half[0]["ps"]
                        kts = range(4, KT_HS)
                    for kt in kts:
                        nc.tensor.matmul(
                            ps[:], w_sb[:, kt, mt * P:(mt + 1) * P],
                            hsT_sb[:, kt, cs],
                            start=(kt == 0), stop=(kt == KT_HS - 1))
                    if half is None or half[1] == 1:
                        if drain == "act":
                            nc.scalar.add(qkvT[:, mt, cs], ps[:],
                                          bq_sb[:, mt:mt + 1])
                        else:
                            nc.vector.tensor_scalar_add(
                                qkvT[:, mt, cs], ps[:], bq_sb[:, mt:mt + 1])

                def v_transposes(pr, nch):
                    # V m-tile (4+pr) 512-col chunk nch covers key tiles
                    # 4nch .. 4nch+3
                    for kt in range(4 * nch, 4 * nch + 4):
                        for hl in range(2):
                            nc.sync.dma_start_transpose(
                                v_sb[:, 2 * pr + hl, kt, 0:HD],
                                qkvT[hl * HD:(hl + 1) * HD, 4 + pr,
                                     kt * P:(kt + 1) * P])

                # lead-in covers only what the first two sections (pair-0,
                # qc-0) need: K-pair0 fully, Q-pair0 qc0 columns, V-pair0.
                for nch in range(4):
                    qkv_chain(2, nch, "act")
                for nch in (0, 1):
                    qkv_chain(0, nch, "act")
                for nch in range(4):
                    qkv_chain(4, nch, "act")
                    v_transposes(0, nch)

                nc.sync.dma_start(wd_sb[:],
                                  wd.rearrange("(t p) n -> p t n", p=P))

                # -------- Phase 2+3: attention + interleaved fill --------
                # fill items: (cost_in_matmuls, kind, ...); ordered by
                # deadline: Q-pair0 qc1 (needed section 3), then K-pair1,
                # Q-pair1 qc0/qc1, V-pair1 (sections 5..8)
                fill = []

                def add_chain(mt, nch):
                    st = {}
                    fill.append((4, "qkv1", mt, nch, (st, 0)))
                    fill.append((4, "qkv1", mt, nch, (st, 1)))

                for nch in (2, 3):
                    add_chain(0, nch)
                for nch in range(4):
                    add_chain(3, nch)
                for nch in (0, 1):
                    add_chain(1, nch)
                for nch in range(4):
                    add_chain(5, nch)
                for nch in (2, 3):
                    add_chain(1, nch)

                def dense_qt(qc, qt):
                    q0 = qc * QC + qt * P
                    y_t = yp.tile([P, D], BF16, tag="y")
                    for ch in range(2):
                        cs = slice(ch * 512, (ch + 1) * 512)
                        ps = sps.tile([P, 512], F32, tag="s",
                                      name=f"d_{qc}_{qt}_{ch}")
                        for pr2 in range(2):
                            nc.tensor.matmul(
                                ps[:], ctx_pair[:, pr2, q0:q0 + P],
                                wd_sb[:, pr2, cs],
                                start=(pr2 == 0), stop=(pr2 == 1))
                        nc.vector.tensor_copy(y_t[:, cs], ps[:])
                    nc.sync.dma_start(y[q0:q0 + P, :], y_t[:])

                credit = [0.0]

                def pop_one():
                    item = fill.pop(0)
                    if item[1] == "qkv1":
                        qkv_chain(item[2], item[3], "dve", item[4])
                        if item[2] == 5 and item[4][1] == 1:
                            v_transposes(1, item[3])
                    else:
                        dense_qt(item[2], item[3])

                def pop_fill(budget):
                    """credit-metered fill: ~budget matmul-equivalents per
                    call keeps each PE fill burst under the exp period"""
                    credit[0] = min(credit[0] + budget, 8.0)
                    while fill and fill[0][0] <= credit[0]:
                        credit[0] -= fill[0][0]
                        pop_one()

                def drain_fill():
                    while fill:
                        pop_one()

                # section order defers pair-1 heads so the pair-1 QKV fill
                # chains get 4 sections of runway instead of 2
                SECTIONS = [(0, 0), (0, 1), (1, 0), (1, 1),
                            (0, 2), (0, 3), (1, 2), (1, 3)]
                masks = {0: mask0}
                for qc, h in SECTIONS:
                    q0 = qc * QC
                    if qc not in masks:
                        mask_t = mp.tile([P, KT_S, QC], BF16, tag="mask")
                        masks[qc] = mask_t
                        for g in range(8):
                            nc.sync.dma_start(
                                mask_t[:, 2 * g:2 * g + 2, :],
                                maskT[g * 256:(g + 1) * 256,
                                      q0:q0 + QC].rearrange(
                                          "(t p) q -> p t q", p=P))
                    mask_t = masks[qc]
                    if True:
                        pr, hl = divmod(h, 2)
                        rows = slice(hl * HD, (hl + 1) * HD)
                        ctx_ps = cps.tile([HD + 1, QC], F32, tag="ctx",
                                          name=f"ctx_{h}_{qc}")
                        # software-pipelined: emit QK(kt+1) before PV(kt) so
                        # the in-order PE stream never blocks the next exp
                        # behind a mask-wait
                        pts = []
                        for kt in range(KT_S):
                            s_ps = sps.tile([P, QC], F32, tag="s",
                                            name=f"s_{h}_{qc}_{kt}")
                            for ch in range(2):
                                cs = slice(ch * 512, (ch + 1) * 512)
                                nc.tensor.matmul(
                                    s_ps[:, cs],
                                    qkvT[rows, 2 + pr, kt * P:(kt + 1) * P],
                                    qkvT[rows, 0 + pr,
                                         q0 + ch * 512:q0 + (ch + 1) * 512],
                                    start=True, stop=True)
                            pt = ptp.tile([P, QC], BF16, tag="pt")
                            nc.scalar.activation(pt[:], s_ps[:], AF.Exp)
                            if kt % 2 == 1:
                                nc.gpsimd.tensor_mul(pt[:], pt[:],
                                                     mask_t[:, kt, :])
                            else:
                                nc.vector.tensor_mul(pt[:], pt[:],
                                                     mask_t[:, kt, :])
                            pts.append(pt)
                            if kt >= 1:
                                pkt = kt - 1
                                for ch in range(2):
                                    cs = slice(ch * 512, (ch + 1) * 512)
                                    nc.tensor.matmul(
                                        ctx_ps[:, cs],
                                        v_sb[:, h, kt - 1, 0:HD + 1],
                                        pts[pkt][:, cs],
                                        start=(pkt == 0), stop=False)
                            pop_fill(1.35)
                        for ch in range(2):
                            cs = slice(ch * 512, (ch + 1) * 512)
                            nc.tensor.matmul(
                                ctx_ps[:, cs],
                                v_sb[:, h, KT_S - 1, 0:HD + 1],
                                pts[KT_S - 1][:, cs],
                                start=False, stop=True)
                        # ---- normalize head h ----
                        # Drain ctx PSUM to SBUF right away (ACT takes the
                        # row-sum row, DVE the 64 ctx rows) so the single
                        # ctx accumulator frees before the next section's
                        # first PV; normalization then runs off-path from
                        # SBUF: k=1 broadcast matmul, 1/x, multiply.
                        rr = rp.tile([HD + 1, QC], BF16, tag="rr")
                        nc.scalar.copy(rr[HD:HD + 1, :],
                                       ctx_ps[HD:HD + 1, :])
                        ctx_sb = rp.tile([HD, QC], F32, tag="ctxs")
                        nc.vector.tensor_copy(ctx_sb[:], ctx_ps[0:HD, :])
                        rb = sps.tile([HD, QC], F32, tag="s",
                                      name=f"rb_{h}_{qc}")
                        for ch in range(2):
                            cs = slice(ch * 512, (ch + 1) * 512)
                            nc.tensor.matmul(
                                rb[:, cs], ones_bf[HD:HD + 1, :],
                                rr[HD:HD + 1, cs], start=True, stop=True)
                        rbi = rp.tile([HD, QC], F32, tag="rbi")
                        nc.vector.reciprocal_approx_fast(rbi[:], rb[:])
                        if hl == 0:
                            nc.vector.tensor_mul(
                                ctx_pair[0:HD, pr, q0:q0 + QC],
                                ctx_sb[:], rbi[:])
                        else:
                            # engines cannot cross partitions: stage on
                            # partitions 0-63, partition-shift with DMA
                            stg = rp.tile([HD, QC], BF16, tag="stg")
                            nc.vector.tensor_mul(stg[:], ctx_sb[:], rbi[:])
                            nc.sync.dma_start(
                                ctx_pair[HD:P, pr, q0:q0 + QC], stg[:])
                    if (qc, h) == (0, 3):
                        fill.extend([(2, "dense", 0, qt)
                                     for qt in range(NQT)])
                drain_fill()
                for qt in range(NQT):
                    dense_qt(1, qt)
                if dbg:
                    nc.sync.dma_start(d_qkvT, qkvT[:])
                    nc.sync.dma_start(d_v, v_sb[:])
                    nc.sync.dma_start(d_cp, ctx_pair[:])

    nc.compile()
    return nc


_NC = None


def get_program():
    global _NC
    if _NC is None:
        _NC = build_program()
    return _NC


def make_in_maps(hidden_states, attention_mask, W_qkv, b_qkv, W_dense, b_dense):
    hs = np.asarray(hidden_states, np.float32)
    mask = np.asarray(attention_mask)
    W_qkv = np.asarray(W_qkv, np.float32)
    b_qkv = np.asarray(b_qkv, np.float32)
    W_dense = np.asarray(W_dense, np.float32)

    hsT = [np.ascontiguousarray(hs[b].T).astype(ml_dtypes.bfloat16)
           for b in range(B)]
    maskT = [np.ascontiguousarray(
        np.where(mask[b, 0], 1.0, 0.0).astype(np.float32).T).astype(
            ml_dtypes.bfloat16) for b in range(B)]

    Wq, Wk, Wv = W_qkv[:, :D], W_qkv[:, D:2 * D], W_qkv[:, 2 * D:]
    bq, bk, bv = b_qkv[:D], b_qkv[D:2 * D], b_qkv[2 * D:]

    in_maps = []
    for c in range(NCORES):
        b = c // (NCORES // B)
        h0 = HPC * (c % (NCORES // B))
        cols = slice(h0 * HD, (h0 + HPC) * HD)
        wqkv_c = np.concatenate(
            [Wq[:, cols] / SCALE, Wk[:, cols], Wv[:, cols]], axis=1)
        bqkv_c = np.concatenate(
            [bq[cols] / SCALE, bk[cols], bv[cols]]).astype(np.float32)
        in_maps.append({
            "hsT": hsT[b],
            "wqkv": np.ascontiguousarray(wqkv_c).astype(ml_dtypes.bfloat16),
            "bqkv": bqkv_c,
            "maskT": maskT[b],
            "wd": np.ascontiguousarray(W_dense[cols, :]).astype(
                ml_dtypes.bfloat16),
        })
    return in_maps


def kernel(hidden_states, attention_mask, W_qkv, b_qkv, W_dense, b_dense,
           **run_kwargs):
    nc = get_program()
    in_maps = make_in_maps(hidden_states, attention_mask, W_qkv, b_qkv,
                           W_dense, b_dense)
    res = bass_utils.run_bass_kernel_spmd(
        nc, in_maps, core_ids=list(range(NCORES)), **run_kwargs)
    out = np.zeros((B, S, D), np.float32)
    gpb = NCORES // B
    for c in range(NCORES):
        out[c // gpb] += np.asarray(res.results[c]["y"], np.float32)
    out += np.asarray(b_dense, np.float32)
    if run_kwargs:
        kernel.last_results = res
    return out


# revision 36
# speedup vs baseline: 1.1118x; 1.1118x over previous
"""Trainium2 Bass kernel for ExpBertSelfAttention (B=2, S=2048, D=1024, H=16).

Sharding: 8 cores; core c handles batch b=c//4 and 4 consecutive heads
4*(c%4)..4*(c%4)+3 (data-parallel on B, tensor-parallel on heads).  The dense
output projection is row-parallel: each core returns a partial [S, D] sum
(bf16); the host adds the 4 partials per batch + b_dense.

Per-core data path (all matmuls bf16 with f32 PSUM; 1/sqrt(hd) folded into
Wq on host; softmax without max-subtraction — scores are O(1) by
construction):

  - QKV: qkvT[768, S] = wqkv^T hsT.  Pair-0 m-tiles run in the lead-in;
    pair-1 runs as 8-matmul accumulation chains interleaved into the
    attention kt loops (PE fill work), both through a shared 3-slot
    [128, 1024] PSUM ring.  Drains: ACT (lead-in) / DVE (in-flight).
  - V is transposed to [key, hd] via XBAR DMA transpose ([64,128]->[128,64]
    into 80-element slots: dst offsets must stay 32B-aligned), with a
    constant ones column at index 64 so PV also emits the softmax row-sum.
  - Attention is processed one head at a time (8 sections = 4 heads x 2
    q-chunks); the ctx accumulator [65, 1024] then needs only 2 PSUM banks.
    Per kt: QK (2 x N=512 matmuls into a ring slot), one [128,1024] exp on
    ACT (psum f32 -> sbuf bf16), multiplicative {0,1} bf16 mask on DVE (2x
    16-bit rate) or GPSIMD (1 in 3, SBUF-only), PV (2 x N=512 into ctx).
  - normalize: row-sum row 64 -> ACT copy to SBUF -> k=1 matmul broadcast
    across partitions (into a ring slot) -> reciprocal_approx_fast (DVE) ->
    multiply into the pair-stacked bf16 ctx_pair; odd heads reach
    partitions 64-127 via a small partition-shifting SBUF->SBUF DMA.
  - dense: y[qtile, D] accumulated over the 2 pairs through ring slots,
    DVE-drained to bf16, streamed out per qtile; the qc-0 half overlaps the
    qc-1 attention.

Engine budget per core (TimelineSim model): PE.ENGINE ~167us (401k rows
@2.4GHz) is the roof; PE.SEQ ~144us (880 Ldweights+Matmult pairs), ACT
~141us (128 exps + row copies), DVE ~107us, Pool ~87us, DMA ~62us fit under.
"""

import os
import sys

for _p in ("/opt/trn_rl_repo", "/root/.axon_site/_ro/trn_rl_repo"):
    if os.path.isdir(_p) and _p not in sys.path:
        sys.path.insert(0, _p)

import numpy as np
import ml_dtypes

import concourse.bass as bass
import concourse.tile as tile
from concourse import bacc, mybir
from concourse import bass_utils

B, S, D, H = 2, 2048, 1024, 16
HD = D // H  # 64
SCALE = float(np.sqrt(HD).astype(np.float32))
NCORES = 8
HPC = H // (NCORES // B)  # heads per core = 4
P = 128
F32 = mybir.dt.float32
BF16 = mybir.dt.bfloat16
AF = mybir.ActivationFunctionType

KT_HS = D // P            # 8 contraction tiles for QKV
KT_S = S // P             # 16 key tiles for attention
QC = 1024                 # q chunk
NQC = S // QC             # 2
NQT = QC // P             # 8 query tiles per chunk
VW = 80                   # v_sb slot stride (65 used: 64 v + ones column);
                          # XBAR-transpose dst offsets must be 32B-aligned


def build_program():
    nc = bacc.Bacc("TRN2", target_bir_lowering=False, debug=False,
                   num_devices=NCORES)

    hsT = nc.dram_tensor("hsT", [D, S], BF16, kind="ExternalInput").ap()
    wqkv = nc.dram_tensor("wqkv", [D, 3 * HPC * HD], BF16,
                          kind="ExternalInput").ap()
    bqkv = nc.dram_tensor("bqkv", [3 * HPC * HD], F32,
                          kind="ExternalInput").ap()
    maskT = nc.dram_tensor("maskT", [S, S], BF16, kind="ExternalInput").ap()
    wd = nc.dram_tensor("wd", [2 * P, D], BF16, kind="ExternalInput").ap()
    y = nc.dram_tensor("y", [S, D], BF16, kind="ExternalOutput").ap()
    dbg = os.environ.get("BK_DEBUG", "") == "1"
    if dbg:
        d_qkvT = nc.dram_tensor("d_qkvT", [P, 6, S], BF16,
                                kind="ExternalOutput").ap()
        d_v = nc.dram_tensor("d_v", [P, HPC, KT_S, VW], BF16,
                             kind="ExternalOutput").ap()
        d_cp = nc.dram_tensor("d_cp", [P, 2, S], BF16,
                              kind="ExternalOutput").ap()

    hsT_r = hsT.rearrange("(t p) n -> p t n", p=P)
    w_r = wqkv.rearrange("(t p) n -> p t n", p=P)

    with tile.TileContext(nc) as tc:
        with tc.tile_pool(name="persist", bufs=1) as persist:
            hsT_sb = persist.tile([P, KT_HS, S], BF16)          # 32 KB/part
            w_sb = persist.tile([P, KT_HS, 3 * HPC * HD], BF16)  # 12 KB/part
            bq_sb = persist.tile([P, 6], F32)
            qkvT = persist.tile([P, 6, S], BF16)                # 24 KB/part
            # v slots [h, kt, VW]: cols 0-63 = V^T, col 64 = ones
            v_sb = persist.tile([P, HPC, KT_S, VW], BF16)       # 10 KB/part
            wd_sb = persist.tile([P, 2, D], BF16)               # 4 KB/part
            ctx_pair = persist.tile([P, 2, S], BF16)            # 8 KB/part
            ones_f = persist.tile([P, HD], F32)
            ones_bf = persist.tile([P, HD], BF16)

            nc.sync.dma_start(bq_sb[:], bqkv.rearrange("(t p) -> p t", p=P))
            nc.vector.memset(ones_f[:], 1.0)
            nc.vector.tensor_copy(ones_bf[:], ones_f[:])
            nc.vector.tensor_copy(
                v_sb[:, :, :, HD:HD + 1].rearrange("p a b c -> p (a b c)"),
                ones_f[:, 0:HPC * KT_S])

            with (
                tc.tile_pool(name="mp", bufs=2) as mp,
                tc.tile_pool(name="ptp", bufs=6) as ptp,
                tc.tile_pool(name="rp", bufs=2) as rp,
                tc.tile_pool(name="yp", bufs=2) as yp,
                tc.tile_pool(name="sps", bufs=3, space="PSUM") as sps,
                tc.tile_pool(name="cps", bufs=1, space="PSUM") as cps,
            ):
                # ---------------- Phase 1: QKV pair 0 ----------------
                # (w_kt, hsT_kt cols 0:512) DMA pairs stream first so the
                # first accumulation chains are DMA-complete early.
                for kt in range(KT_HS):
                    nc.sync.dma_start(w_sb[:, kt, :], w_r[:, kt, :])
                    nc.sync.dma_start(hsT_sb[:, kt, 0:512],
                                      hsT_r[:, kt, 0:512])
                # first key tiles of the qc-0 mask sneak in ahead of the
                # hsT second halves so attention isn't mask-gated at start
                mask0 = mp.tile([P, KT_S, QC], BF16, tag="mask",
                                name="mask_qc0")
                for g in range(2):
                    nc.sync.dma_start(
                        mask0[:, 2 * g:2 * g + 2, :],
                        maskT[g * 256:(g + 1) * 256, 0:QC].rearrange(
                            "(t p) q -> p t q", p=P))
                for kt in range(KT_HS):
                    nc.sync.dma_start(hsT_sb[:, kt, 512:S],
                                      hsT_r[:, kt, 512:S])
                for g in range(2, 8):
                    nc.sync.dma_start(
                        mask0[:, 2 * g:2 * g + 2, :],
                        maskT[g * 256:(g + 1) * 256, 0:QC].rearrange(
                            "(t p) q -> p t q", p=P))

                def qkv_chain(mt, nch, drain, half=None):
                    """one [128,512] column chunk of qkvT m-tile mt through
                    a ring slot; drain = 'act' (lead-in) or 'dve'.
                    half: None = whole 8-kt chain; (state, 0)/(state, 1) =
                    4-kt halves (keeps each PE burst under the exp period
                    when interleaved into attention)."""
                    cs = slice(nch * 512, (nch + 1) * 512)
                    if half is None or half[1] == 0:
                        ps = sps.tile([P, 512], F32, tag="s",
                                      name=f"qkv_ps{mt}_{nch}")
                        if half is not None:
                            half[0]["ps"] = ps
                        kts = range(KT_HS) if half is None else range(4)
                    else:
                        ps = half[0]["ps"]
                        kts = range(4, KT_HS)
                    for kt in kts:
                        nc.tensor.matmul(
                            ps[:], w_sb[:, kt, mt * P:(mt + 1) * P],
                            hsT_sb[:, kt, cs],
                            start=(kt == 0), stop=(kt == KT_HS - 1))
                    if half is None or half[1] == 1:
                        if drain == "act":
                            nc.scalar.add(qkvT[:, mt, cs], ps[:],
                                          bq_sb[:, mt:mt + 1])
                        else:
                            nc.vector.tensor_scalar_add(
                                qkvT[:, mt, cs], ps[:], bq_sb[:, mt:mt + 1])

                def v_transposes(pr, nch):
                    # V m-tile (4+pr) 512-col chunk nch covers key tiles
                    # 4nch .. 4nch+3
                    for kt in range(4 * nch, 4 * nch + 4):
                        for hl in range(2):
                            nc.sync.dma_start_transpose(
                                v_sb[:, 2 * pr + hl, kt, 0:HD],
                                qkvT[hl * HD:(hl + 1) * HD, 4 + pr,
                                     kt * P:(kt + 1) * P])

                # lead-in covers only what the first two sections (pair-0,
                # qc-0) need: K-pair0 fully, Q-pair0 qc0 columns, V-pair0 —
                # ordered per-nch to match hsT DMA arrival.
                for nch in range(4):
                    qkv_chain(2, nch, "act")
                    if nch < 2:
                        qkv_chain(0, nch, "act")
                    qkv_chain(4, nch, "act")
                    v_transposes(0, nch)

                nc.sync.dma_start(wd_sb[:],
                                  wd.rearrange("(t p) n -> p t n", p=P))
                # qc-1 mask loads dispatch now (no deps): the SP queue is
                # in-order, so emitting them later would head-of-line block
                # behind per-section stg DMAs
                mask1 = mp.tile([P, KT_S, QC], BF16, tag="mask",
                                name="mask_qc1")
                for g in range(8):
                    nc.sync.dma_start(
                        mask1[:, 2 * g:2 * g + 2, :],
                        maskT[g * 256:(g + 1) * 256, QC:2 * QC].rearrange(
                            "(t p) q -> p t q", p=P))

                # -------- Phase 2+3: attention + interleaved fill --------
                # fill items: (cost_in_matmuls, kind, ...); ordered by
                # deadline: Q-pair0 qc1 (needed section 3), then K-pair1,
                # Q-pair1 qc0/qc1, V-pair1 (sections 5..8)
                fill = []

                def add_chain(mt, nch):
                    st = {}
                    fill.append((4, "qkv1", mt, nch, (st, 0)))
                    fill.append((4, "qkv1", mt, nch, (st, 1)))

                for nch in (2, 3):
                    add_chain(0, nch)
                for nch in range(4):
                    add_chain(3, nch)
                for nch in (0, 1):
                    add_chain(1, nch)
                for nch in range(4):
                    add_chain(5, nch)
                for nch in (2, 3):
                    add_chain(1, nch)

                def dense_qt(qc, qt):
                    q0 = qc * QC + qt * P
                    y_t = yp.tile([P, D], BF16, tag="y")
                    for ch in range(2):
                        cs = slice(ch * 512, (ch + 1) * 512)
                        ps = sps.tile([P, 512], F32, tag="s",
                                      name=f"d_{qc}_{qt}_{ch}")
                        for pr2 in range(2):
                            nc.tensor.matmul(
                                ps[:], ctx_pair[:, pr2, q0:q0 + P],
                                wd_sb[:, pr2, cs],
                                start=(pr2 == 0), stop=(pr2 == 1))
                        nc.vector.tensor_copy(y_t[:, cs], ps[:])
                    nc.sync.dma_start(y[q0:q0 + P, :], y_t[:])

                credit = [0.0]

                def pop_one():
                    item = fill.pop(0)
                    if item[1] == "qkv1":
                        qkv_chain(item[2], item[3], "dve", item[4])
                        if item[2] == 5 and item[4][1] == 1:
                            v_transposes(1, item[3])
                    else:
                        dense_qt(item[2], item[3])

                def pop_fill(budget):
                    """credit-metered fill: ~budget matmul-equivalents per
                    call keeps each PE fill burst under the exp period"""
                    credit[0] = min(credit[0] + budget, 8.0)
                    while fill and fill[0][0] <= credit[0]:
                        credit[0] -= fill[0][0]
                        pop_one()

                def drain_fill():
                    while fill:
                        pop_one()

                # section order defers pair-1 heads so the pair-1 QKV fill
                # chains get 4 sections of runway instead of 2
                SECTIONS = [(0, 0), (0, 1), (1, 0), (1, 1),
                            (0, 2), (0, 3), (1, 2), (1, 3)]
                masks = {0: mask0, 1: mask1}
                # deferred final-PV + normalize, emitted 1-2 kt into the
                # NEXT section so the boundary never blocks the PE stream
                pending = [None]

                def flush_pending():
                    if pending[0] is None:
                        return
                    qc, h, ctx_ps, pt15 = pending[0]
                    pending[0] = None
                    pr, hl = divmod(h, 2)
                    q0 = qc * QC
                    for ch in range(2):
                        cs = slice(ch * 512, (ch + 1) * 512)
                        nc.tensor.matmul(
                            ctx_ps[:, cs], v_sb[:, h, KT_S - 1, 0:HD + 1],
                            pt15[:, cs], start=False, stop=True)
                    # drain ctx PSUM to SBUF right away (ACT row-sum row,
                    # DVE ctx rows) so the single ctx accumulator frees for
                    # this section's first PV; normalization then runs
                    # off-path from SBUF.
                    rr = rp.tile([HD + 1, QC], BF16, tag="rr",
                                 name=f"rr_{h}_{qc}")
                    nc.scalar.copy(rr[HD:HD + 1, :], ctx_ps[HD:HD + 1, :])
                    ctx_sb = rp.tile([HD, QC], F32, tag="ctxs",
                                     name=f"ctxs_{h}_{qc}")
                    nc.vector.tensor_copy(ctx_sb[:], ctx_ps[0:HD, :])
                    rb = sps.tile([HD, QC], F32, tag="s",
                                  name=f"rb_{h}_{qc}")
                    for ch in range(2):
                        cs = slice(ch * 512, (ch + 1) * 512)
                        nc.tensor.matmul(
                            rb[:, cs], ones_bf[HD:HD + 1, :],
                            rr[HD:HD + 1, cs], start=True, stop=True)
                    rbi = rp.tile([HD, QC], F32, tag="rbi",
                                  name=f"rbi_{h}_{qc}")
                    nc.vector.reciprocal_approx_fast(rbi[:], rb[:])
                    if hl == 0:
                        nc.vector.tensor_mul(
                            ctx_pair[0:HD, pr, q0:q0 + QC], ctx_sb[:],
                            rbi[:])
                    else:
                        # engines cannot cross partitions: stage on
                        # partitions 0-63, partition-shift with DMA
                        stg = rp.tile([HD, QC], BF16, tag="stg",
                                      name=f"stg_{h}_{qc}")
                        nc.vector.tensor_mul(stg[:], ctx_sb[:], rbi[:])
                        nc.sync.dma_start(
                            ctx_pair[HD:P, pr, q0:q0 + QC], stg[:])
                    if (qc, h) == (0, 3):
                        fill.extend([(2, "dense", 0, qt)
                                     for qt in range(NQT)])

                for qc, h in SECTIONS:
                    q0 = qc * QC
                    mask_t = masks[qc]
                    pr, hl = divmod(h, 2)
                    rows = slice(hl * HD, (hl + 1) * HD)
                    ctx_ps = cps.tile([HD + 1, QC], F32, tag="ctx",
                                      name=f"ctx_{h}_{qc}")
                    # software-pipelined: emit QK(kt+1) before PV(kt) so
                    # the in-order PE stream never blocks the next exp
                    # behind a mask-wait
                    pts = []
                    for kt in range(KT_S):
                        s_ps = sps.tile([P, QC], F32, tag="s",
                                        name=f"s_{h}_{qc}_{kt}")
                        for ch in range(2):
                            cs = slice(ch * 512, (ch + 1) * 512)
                            nc.tensor.matmul(
                                s_ps[:, cs],
                                qkvT[rows, 2 + pr, kt * P:(kt + 1) * P],
                                qkvT[rows, 0 + pr,
                                     q0 + ch * 512:q0 + (ch + 1) * 512],
                                start=True, stop=True)
                        pt = ptp.tile([P, QC], BF16, tag="pt")
                        nc.scalar.activation(pt[:], s_ps[:], AF.Exp)
                        if kt % 2 == 1:
                            nc.gpsimd.tensor_mul(pt[:], pt[:],
                                                 mask_t[:, kt, :])
                        else:
                            nc.vector.tensor_mul(pt[:], pt[:],
                                                 mask_t[:, kt, :])
                        pts.append(pt)
                        if kt == 1:
                            flush_pending()
                        if kt >= 1:
                            pkt = kt - 1
                            for ch in range(2):
                                cs = slice(ch * 512, (ch + 1) * 512)
                                nc.tensor.matmul(
                                    ctx_ps[:, cs],
                                    v_sb[:, h, kt - 1, 0:HD + 1],
                                    pts[pkt][:, cs],
                                    start=(pkt == 0), stop=False)
                        pop_fill(1.35)
                    pending[0] = (qc, h, ctx_ps, pts[KT_S - 1])
                flush_pending()
                drain_fill()
                for qt in range(NQT):
                    dense_qt(1, qt)
                if dbg:
                    nc.sync.dma_start(d_qkvT, qkvT[:])
                    nc.sync.dma_start(d_v, v_sb[:])
                    nc.sync.dma_start(d_cp, ctx_pair[:])

    nc.compile()
    return nc


_NC = None


def get_program():
    global _NC
    if _NC is None:
        _NC = build_program()
    return _NC


def make_in_maps(hidden_states, attention_mask, W_qkv, b_qkv, W_dense, b_dense):
    hs = np.asarray(hidden_states, np.float32)
    mask = np.asarray(attention_mask)
    W_qkv = np.asarray(W_qkv, np.float32)
    b_qkv = np.asarray(b_qkv, np.float32)
    W_dense = np.asarray(W_dense, np.float32)

    hsT = [np.ascontiguousarray(hs[b].T).astype(ml_dtypes.bfloat16)
           for b in range(B)]
    maskT = [np.ascontiguousarray(
        np.where(mask[b, 0], 1.0, 0.0).astype(np.float32).T).astype(
            ml_dtypes.bfloat16) for b in range(B)]

    Wq, Wk, Wv = W_qkv[:, :D], W_qkv[:, D:2 * D], W_qkv[:, 2 * D:]
    bq, bk, bv = b_qkv[:D], b_qkv[D:2 * D], b_qkv[2 * D:]

    in_maps = []
    for c in range(NCORES):
        b = c // (NCORES // B)
        h0 = HPC * (c % (NCORES // B))
        cols = slice(h0 * HD, (h0 + HPC) * HD)
        wqkv_c = np.concatenate(
            [Wq[:, cols] / SCALE, Wk[:, cols], Wv[:, cols]], axis=1)
        bqkv_c = np.concatenate(
            [bq[cols] / SCALE, bk[cols], bv[cols]]).astype(np.float32)
        in_maps.append({
            "hsT": hsT[b],
            "wqkv": np.ascontiguousarray(wqkv_c).astype(ml_dtypes.bfloat16),
            "bqkv": bqkv_c,
            "maskT": maskT[b],
            "wd": np.ascontiguousarray(W_dense[cols, :]).astype(
                ml_dtypes.bfloat16),
        })
    return in_maps


def kernel(hidden_states, attention_mask, W_qkv, b_qkv, W_dense, b_dense,
           **run_kwargs):
    nc = get_program()
    in_maps = make_in_maps(hidden_states, attention_mask, W_qkv, b_qkv,
                           W_dense, b_dense)
    res = bass_utils.run_bass_kernel_spmd(
        nc, in_maps, core_ids=list(range(NCORES)), **run_kwargs)
    out = np.zeros((B, S, D), np.float32)
    gpb = NCORES // B
    for c in range(NCORES):
        out[c // gpb] += np.asarray(res.results[c]["y"], np.float32)
    out += np.asarray(b_dense, np.float32)
    if run_kwargs:
        kernel.last_results = res
    return out


# revision 38
# speedup vs baseline: 1.1295x; 1.0159x over previous
"""Trainium2 Bass kernel for ExpBertSelfAttention (B=2, S=2048, D=1024, H=16).

Sharding: 8 cores; core c handles batch b=c//4 and 4 consecutive heads
4*(c%4)..4*(c%4)+3 (data-parallel on B, tensor-parallel on heads).  The dense
output projection is row-parallel: each core returns a partial [S, D] sum
(bf16); the host adds the 4 partials per batch + b_dense.

Per-core data path (all matmuls bf16 with f32 PSUM; 1/sqrt(hd) folded into
Wq on host; softmax without max-subtraction — scores are O(1) by
construction):

  - QKV: qkvT[768, S] = wqkv^T hsT.  Pair-0 m-tiles run in the lead-in;
    pair-1 runs as 8-matmul accumulation chains interleaved into the
    attention kt loops (PE fill work), both through a shared 3-slot
    [128, 1024] PSUM ring.  Drains: ACT (lead-in) / DVE (in-flight).
  - V is transposed to [key, hd] via XBAR DMA transpose ([64,128]->[128,64]
    into 80-element slots: dst offsets must stay 32B-aligned), with a
    constant ones column at index 64 so PV also emits the softmax row-sum.
  - Attention is processed one head at a time (8 sections = 4 heads x 2
    q-chunks); the ctx accumulator [65, 1024] then needs only 2 PSUM banks.
    Per kt: QK (2 x N=512 matmuls into a ring slot), one [128,1024] exp on
    ACT (psum f32 -> sbuf bf16), multiplicative {0,1} bf16 mask on DVE (2x
    16-bit rate) or GPSIMD (1 in 3, SBUF-only), PV (2 x N=512 into ctx).
  - normalize: row-sum row 64 -> ACT copy to SBUF -> k=1 matmul broadcast
    across partitions (into a ring slot) -> reciprocal_approx_fast (DVE) ->
    multiply into the pair-stacked bf16 ctx_pair; odd heads reach
    partitions 64-127 via a small partition-shifting SBUF->SBUF DMA.
  - dense: y[qtile, D] accumulated over the 2 pairs through ring slots,
    DVE-drained to bf16, streamed out per qtile; the qc-0 half overlaps the
    qc-1 attention.

Engine budget per core (TimelineSim model): PE.ENGINE ~167us (401k rows
@2.4GHz) is the roof; PE.SEQ ~144us (880 Ldweights+Matmult pairs), ACT
~141us (128 exps + row copies), DVE ~107us, Pool ~87us, DMA ~62us fit under.
"""

import os
import sys

for _p in ("/opt/trn_rl_repo", "/root/.axon_site/_ro/trn_rl_repo"):
    if os.path.isdir(_p) and _p not in sys.path:
        sys.path.insert(0, _p)

import numpy as np
import ml_dtypes

import concourse.bass as bass
import concourse.tile as tile
from concourse import bacc, mybir
from concourse import bass_utils

B, S, D, H = 2, 2048, 1024, 16
HD = D // H  # 64
SCALE = float(np.sqrt(HD).astype(np.float32))
NCORES = 8
HPC = H // (NCORES // B)  # heads per core = 4
P = 128
F32 = mybir.dt.float32
BF16 = mybir.dt.bfloat16
AF = mybir.ActivationFunctionType

KT_HS = D // P            # 8 contraction tiles for QKV
KT_S = S // P             # 16 key tiles for attention
QC = 1024                 # q chunk
NQC = S // QC             # 2
NQT = QC // P             # 8 query tiles per chunk
VW = 80                   # v_sb slot stride (65 used: 64 v + ones column);
                          # XBAR-transpose dst offsets must be 32B-aligned


def build_program():
    nc = bacc.Bacc("TRN2", target_bir_lowering=False, debug=False,
                   num_devices=NCORES)

    hsT = nc.dram_tensor("hsT", [D, S], BF16, kind="ExternalInput").ap()
    wqkv = nc.dram_tensor("wqkv", [D, 3 * HPC * HD], BF16,
                          kind="ExternalInput").ap()
    bqkv = nc.dram_tensor("bqkv", [3 * HPC * HD], F32,
                          kind="ExternalInput").ap()
    maskT = nc.dram_tensor("maskT", [S, S], BF16, kind="ExternalInput").ap()
    wd = nc.dram_tensor("wd", [2 * P, D], BF16, kind="ExternalInput").ap()
    y = nc.dram_tensor("y", [S, D], BF16, kind="ExternalOutput").ap()
    dbg = os.environ.get("BK_DEBUG", "") == "1"
    if dbg:
        d_qkvT = nc.dram_tensor("d_qkvT", [P, 6, S], BF16,
                                kind="ExternalOutput").ap()
        d_v = nc.dram_tensor("d_v", [P, HPC, KT_S, VW], BF16,
                             kind="ExternalOutput").ap()
        d_cp = nc.dram_tensor("d_cp", [P, 2, S], BF16,
                              kind="ExternalOutput").ap()

    hsT_r = hsT.rearrange("(t p) n -> p t n", p=P)
    w_r = wqkv.rearrange("(t p) n -> p t n", p=P)

    with tile.TileContext(nc) as tc:
        with tc.tile_pool(name="persist", bufs=1) as persist:
            hsT_sb = persist.tile([P, KT_HS, S], BF16)          # 32 KB/part
            w_sb = persist.tile([P, KT_HS, 3 * HPC * HD], BF16)  # 12 KB/part
            bq_sb = persist.tile([P, 6], F32)
            qkvT = persist.tile([P, 6, S], BF16)                # 24 KB/part
            # v slots [h, kt, VW]: cols 0-63 = V^T, col 64 = ones
            v_sb = persist.tile([P, HPC, KT_S, VW], BF16)       # 10 KB/part
            wd_sb = persist.tile([P, 2, D], BF16)               # 4 KB/part
            ctx_pair = persist.tile([P, 2, S], BF16)            # 8 KB/part
            ones_f = persist.tile([P, HD], F32)
            ones_bf = persist.tile([P, HD], BF16)

            nc.sync.dma_start(bq_sb[:], bqkv.rearrange("(t p) -> p t", p=P))
            nc.vector.memset(ones_f[:], 1.0)
            nc.vector.tensor_copy(ones_bf[:], ones_f[:])
            nc.vector.tensor_copy(
                v_sb[:, :, :, HD:HD + 1].rearrange("p a b c -> p (a b c)"),
                ones_f[:, 0:HPC * KT_S])

            with (
                tc.tile_pool(name="mp", bufs=2) as mp,
                tc.tile_pool(name="ptp", bufs=6) as ptp,
                tc.tile_pool(name="rp", bufs=2) as rp,
                tc.tile_pool(name="yp", bufs=2) as yp,
                tc.tile_pool(name="sps", bufs=3, space="PSUM") as sps,
                tc.tile_pool(name="cps", bufs=1, space="PSUM") as cps,
            ):
                # ---------------- Phase 1: QKV pair 0 ----------------
                # (w_kt, hsT_kt cols 0:512) DMA pairs stream first so the
                # first accumulation chains are DMA-complete early.
                for kt in range(KT_HS):
                    nc.sync.dma_start(w_sb[:, kt, :], w_r[:, kt, :])
                    nc.sync.dma_start(hsT_sb[:, kt, 0:512],
                                      hsT_r[:, kt, 0:512])
                # first key tiles of the qc-0 mask sneak in ahead of the
                # hsT second halves so attention isn't mask-gated at start
                mask0 = mp.tile([P, KT_S, QC], BF16, tag="mask",
                                name="mask_qc0")
                for g in range(2):
                    nc.sync.dma_start(
                        mask0[:, 2 * g:2 * g + 2, :],
                        maskT[g * 256:(g + 1) * 256, 0:QC].rearrange(
                            "(t p) q -> p t q", p=P))
                for kt in range(KT_HS):
                    nc.sync.dma_start(hsT_sb[:, kt, 512:S],
                                      hsT_r[:, kt, 512:S])
                for g in range(2, 8):
                    nc.sync.dma_start(
                        mask0[:, 2 * g:2 * g + 2, :],
                        maskT[g * 256:(g + 1) * 256, 0:QC].rearrange(
                            "(t p) q -> p t q", p=P))

                def qkv_chain(mt, nch, drain, half=None):
                    """one [128,512] column chunk of qkvT m-tile mt through
                    a ring slot; drain = 'act' (lead-in) or 'dve'.
                    half: None = whole 8-kt chain; (state, 0)/(state, 1) =
                    4-kt halves (keeps each PE burst under the exp period
                    when interleaved into attention)."""
                    cs = slice(nch * 512, (nch + 1) * 512)
                    if half is None or half[1] == 0:
                        ps = sps.tile([P, 512], F32, tag="s",
                                      name=f"qkv_ps{mt}_{nch}")
                        if half is not None:
                            half[0]["ps"] = ps
                        kts = range(KT_HS) if half is None else range(4)
                    else:
                        ps = half[0]["ps"]
                        kts = range(4, KT_HS)
                    for kt in kts:
                        nc.tensor.matmul(
                            ps[:], w_sb[:, kt, mt * P:(mt + 1) * P],
                            hsT_sb[:, kt, cs],
                            start=(kt == 0), stop=(kt == KT_HS - 1))
                    if half is None or half[1] == 1:
                        if drain == "act":
                            nc.scalar.add(qkvT[:, mt, cs], ps[:],
                                          bq_sb[:, mt:mt + 1])
                        else:
                            nc.vector.tensor_scalar_add(
                                qkvT[:, mt, cs], ps[:], bq_sb[:, mt:mt + 1])

                def v_transposes(pr, nch):
                    # V m-tile (4+pr) 512-col chunk nch covers key tiles
                    # 4nch .. 4nch+3
                    for kt in range(4 * nch, 4 * nch + 4):
                        for hl in range(2):
                            nc.sync.dma_start_transpose(
                                v_sb[:, 2 * pr + hl, kt, 0:HD],
                                qkvT[hl * HD:(hl + 1) * HD, 4 + pr,
                                     kt * P:(kt + 1) * P])

                # lead-in covers only what the first two sections (pair-0,
                # qc-0) need: K-pair0 fully, Q-pair0 qc0 columns, V-pair0 —
                # ordered per-nch to match hsT DMA arrival.
                for nch in range(4):
                    qkv_chain(2, nch, "act")
                    if nch < 2:
                        qkv_chain(0, nch, "act")
                    qkv_chain(4, nch, "act")
                    v_transposes(0, nch)

                nc.sync.dma_start(wd_sb[:],
                                  wd.rearrange("(t p) n -> p t n", p=P))
                # qc-1 mask loads dispatch now (no deps): the SP queue is
                # in-order, so emitting them later would head-of-line block
                # behind per-section stg DMAs
                mask1 = mp.tile([P, KT_S, QC], BF16, tag="mask",
                                name="mask_qc1")
                for g in range(8):
                    nc.sync.dma_start(
                        mask1[:, 2 * g:2 * g + 2, :],
                        maskT[g * 256:(g + 1) * 256, QC:2 * QC].rearrange(
                            "(t p) q -> p t q", p=P))

                # -------- Phase 2+3: attention + interleaved fill --------
                # fill items: (cost_in_matmuls, kind, ...); ordered by
                # deadline: Q-pair0 qc1 (needed section 3), then K-pair1,
                # Q-pair1 qc0/qc1, V-pair1 (sections 5..8)
                fill = []

                def add_chain(mt, nch):
                    st = {}
                    fill.append((4, "qkv1", mt, nch, (st, 0)))
                    fill.append((4, "qkv1", mt, nch, (st, 1)))

                for nch in (2, 3):
                    add_chain(0, nch)
                for nch in range(4):
                    add_chain(3, nch)
                for nch in (0, 1):
                    add_chain(1, nch)
                for nch in range(4):
                    add_chain(5, nch)
                for nch in (2, 3):
                    add_chain(1, nch)

                def dense_qt(qc, qt):
                    q0 = qc * QC + qt * P
                    y_t = yp.tile([P, D], BF16, tag="y")
                    for ch in range(2):
                        cs = slice(ch * 512, (ch + 1) * 512)
                        ps = sps.tile([P, 512], F32, tag="s",
                                      name=f"d_{qc}_{qt}_{ch}")
                        for pr2 in range(2):
                            nc.tensor.matmul(
                                ps[:], ctx_pair[:, pr2, q0:q0 + P],
                                wd_sb[:, pr2, cs],
                                start=(pr2 == 0), stop=(pr2 == 1))
                        nc.vector.tensor_copy(y_t[:, cs], ps[:])
                    nc.sync.dma_start(y[q0:q0 + P, :], y_t[:])

                credit = [0.0]

                def pop_one():
                    item = fill.pop(0)
                    if item[1] == "qkv1":
                        qkv_chain(item[2], item[3], "dve", item[4])
                        if item[2] == 5 and item[4][1] == 1:
                            v_transposes(1, item[3])
                    else:
                        dense_qt(item[2], item[3])

                def pop_fill(budget):
                    """credit-metered fill: ~budget matmul-equivalents per
                    call keeps each PE fill burst under the exp period"""
                    credit[0] = min(credit[0] + budget, 8.0)
                    while fill and fill[0][0] <= credit[0]:
                        credit[0] -= fill[0][0]
                        pop_one()

                def drain_fill():
                    while fill:
                        pop_one()

                # section order defers pair-1 heads so the pair-1 QKV fill
                # chains get 4 sections of runway instead of 2
                SECTIONS = [(0, 0), (0, 1), (1, 0), (1, 1),
                            (0, 2), (0, 3), (1, 2), (1, 3)]
                masks = {0: mask0, 1: mask1}
                # deferred final-PV + normalize, emitted 1-2 kt into the
                # NEXT section so the boundary never blocks the PE stream
                pending = [None]

                def flush_pending():
                    if pending[0] is None:
                        return
                    qc, h, ctx_ps, pt15 = pending[0]
                    pending[0] = None
                    pr, hl = divmod(h, 2)
                    q0 = qc * QC
                    for ch in range(2):
                        cs = slice(ch * 512, (ch + 1) * 512)
                        nc.tensor.matmul(
                            ctx_ps[:, cs], v_sb[:, h, KT_S - 1, 0:HD + 1],
                            pt15[:, cs], start=False, stop=True)
                    # drain ctx PSUM to SBUF right away (ACT row-sum row,
                    # DVE ctx rows) so the single ctx accumulator frees for
                    # this section's first PV; normalization then runs
                    # off-path from SBUF.
                    rr = rp.tile([HD + 1, QC], BF16, tag="rr",
                                 name=f"rr_{h}_{qc}")
                    nc.scalar.copy(rr[HD:HD + 1, :], ctx_ps[HD:HD + 1, :])
                    ctx_sb = rp.tile([HD, QC], F32, tag="ctxs",
                                     name=f"ctxs_{h}_{qc}")
                    nc.vector.tensor_copy(ctx_sb[:], ctx_ps[0:HD, :])
                    rb = sps.tile([HD, QC], F32, tag="s",
                                  name=f"rb_{h}_{qc}")
                    for ch in range(2):
                        cs = slice(ch * 512, (ch + 1) * 512)
                        nc.tensor.matmul(
                            rb[:, cs], ones_bf[HD:HD + 1, :],
                            rr[HD:HD + 1, cs], start=True, stop=True)
                    rbi = rp.tile([HD, QC], F32, tag="rbi",
                                  name=f"rbi_{h}_{qc}")
                    nc.vector.reciprocal_approx_fast(rbi[:], rb[:])
                    if hl == 0:
                        nc.vector.tensor_mul(
                            ctx_pair[0:HD, pr, q0:q0 + QC], ctx_sb[:],
                            rbi[:])
                    else:
                        # engines cannot cross partitions: stage on
                        # partitions 0-63, partition-shift with DMA
                        stg = rp.tile([HD, QC], BF16, tag="stg",
                                      name=f"stg_{h}_{qc}")
                        nc.vector.tensor_mul(stg[:], ctx_sb[:], rbi[:])
                        nc.sync.dma_start(
                            ctx_pair[HD:P, pr, q0:q0 + QC], stg[:])

                for qc, h in SECTIONS:
                    q0 = qc * QC
                    mask_t = masks[qc]
                    pr, hl = divmod(h, 2)
                    rows = slice(hl * HD, (hl + 1) * HD)
                    ctx_ps = cps.tile([HD + 1, QC], F32, tag="ctx",
                                      name=f"ctx_{h}_{qc}")
                    # software-pipelined: emit QK(kt+1) before PV(kt) so
                    # the in-order PE stream never blocks the next exp
                    # behind a mask-wait
                    pts = []
                    for kt in range(KT_S):
                        s_ps = sps.tile([P, QC], F32, tag="s",
                                        name=f"s_{h}_{qc}_{kt}")
                        for ch in range(2):
                            cs = slice(ch * 512, (ch + 1) * 512)
                            nc.tensor.matmul(
                                s_ps[:, cs],
                                qkvT[rows, 2 + pr, kt * P:(kt + 1) * P],
                                qkvT[rows, 0 + pr,
                                     q0 + ch * 512:q0 + (ch + 1) * 512],
                                start=True, stop=True)
                        pt = ptp.tile([P, QC], BF16, tag="pt")
                        nc.scalar.activation(pt[:], s_ps[:], AF.Exp)
                        if kt % 2 == 1:
                            nc.gpsimd.tensor_mul(pt[:], pt[:],
                                                 mask_t[:, kt, :])
                        else:
                            nc.vector.tensor_mul(pt[:], pt[:],
                                                 mask_t[:, kt, :])
                        pts.append(pt)
                        if kt == 1:
                            flush_pending()
                        # dense qc-0 joins the fill queue only once its
                        # ctx_pair inputs are (about to be) complete — a
                        # not-yet-ready chain in the PSUM ring would block
                        # the ring rotation for the attention QKs behind it
                        if (qc, h) == SECTIONS[6] and kt == 6:
                            fill.extend([(2, "dense", 0, qt)
                                         for qt in range(NQT)])
                        if kt >= 1:
                            pkt = kt - 1
                            for ch in range(2):
                                cs = slice(ch * 512, (ch + 1) * 512)
                                nc.tensor.matmul(
                                    ctx_ps[:, cs],
                                    v_sb[:, h, kt - 1, 0:HD + 1],
                                    pts[pkt][:, cs],
                                    start=(pkt == 0), stop=False)
                        pop_fill(1.35)
                    pending[0] = (qc, h, ctx_ps, pts[KT_S - 1])
                flush_pending()
                drain_fill()
                for qt in range(NQT):
                    dense_qt(1, qt)
                if dbg:
                    nc.sync.dma_start(d_qkvT, qkvT[:])
                    nc.sync.dma_start(d_v, v_sb[:])
                    nc.sync.dma_start(d_cp, ctx_pair[:])

    nc.compile()
    return nc


_NC = None


def get_program():
    global _NC
    if _NC is None:
        _NC = build_program()
    return _NC


def make_in_maps(hidden_states, attention_mask, W_qkv, b_qkv, W_dense, b_dense):
    hs = np.asarray(hidden_states, np.float32)
    mask = np.asarray(attention_mask)
    W_qkv = np.asarray(W_qkv, np.float32)
    b_qkv = np.asarray(b_qkv, np.float32)
    W_dense = np.asarray(W_dense, np.float32)

    hsT = [np.ascontiguousarray(hs[b].T).astype(ml_dtypes.bfloat16)
           for b in range(B)]
    maskT = [np.ascontiguousarray(
        np.where(mask[b, 0], 1.0, 0.0).astype(np.float32).T).astype(
            ml_dtypes.bfloat16) for b in range(B)]

    Wq, Wk, Wv = W_qkv[:, :D], W_qkv[:, D:2 * D], W_qkv[:, 2 * D:]
    bq, bk, bv = b_qkv[:D], b_qkv[D:2 * D], b_qkv[2 * D:]

    in_maps = []
    for c in range(NCORES):
        b = c // (NCORES // B)
        h0 = HPC * (c % (NCORES // B))
        cols = slice(h0 * HD, (h0 + HPC) * HD)
        wqkv_c = np.concatenate(
            [Wq[:, cols] / SCALE, Wk[:, cols], Wv[:, cols]], axis=1)
        bqkv_c = np.concatenate(
            [bq[cols] / SCALE, bk[cols], bv[cols]]).astype(np.float32)
        in_maps.append({
            "hsT": hsT[b],
            "wqkv": np.ascontiguousarray(wqkv_c).astype(ml_dtypes.bfloat16),
            "bqkv": bqkv_c,
            "maskT": maskT[b],
            "wd": np.ascontiguousarray(W_dense[cols, :]).astype(
                ml_dtypes.bfloat16),
        })
    return in_maps


def kernel(hidden_states, attention_mask, W_qkv, b_qkv, W_dense, b_dense,
           **run_kwargs):
    nc = get_program()
    in_maps = make_in_maps(hidden_states, attention_mask, W_qkv, b_qkv,
                           W_dense, b_dense)
    res = bass_utils.run_bass_kernel_spmd(
        nc, in_maps, core_ids=list(range(NCORES)), **run_kwargs)
    out = np.zeros((B, S, D), np.float32)
    gpb = NCORES // B
    for c in range(NCORES):
        out[c // gpb] += np.asarray(res.results[c]["y"], np.float32)
    out += np.asarray(b_dense, np.float32)
    if run_kwargs:
        kernel.last_results = res
    return out
